# revision 2
# baseline (speedup 1.0000x reference)
"""Multi-head attention (RoPE-full-dmodel variant) on 8 TRN2 NeuronCores.

Sharding: core c = (batch c//4, head-group c%4 of 4 heads).
 - W_q/W_k/W_v split column-wise by head (each core projects its 256 channels)
 - W_o split row-wise; per-core partial outputs summed on host

Pipelined design (v2):
 - ACT exp is the hard floor: 4 heads x 2048^2 = 16.8M exps/core at 128
   lanes / 1.2 GHz ~= 147us with [128,1024] tiles. Everything else overlaps it.
 - scores MMs are 2-head row-packed: DK=64 contraction -> head A on PE rows
   0-63, head B on rows 64-127, concurrent via tile_position auto-derive.
 - V is projected directly seq-major (vT chunk stationary, wv moving): no PE
   transposes. V bias is folded into V itself (U + bv*denom normalizes to
   U/denom + bv), so normalize is just recip + broadcast + mul.
 - RoPE + Q/K projections stream per 512-seq block so attention starts early;
   V-proj tiles 8-15, Q blocks 1-3 and out-proj are drip-fed into the
   attention emission stream ("background thunks") to fill PE slack without
   starving ACT.
 - exp tile = [128 keys, headA 512q | headB 512q]; PV accumulates into
   [65, 512] psums (64 dk rows + denominator row from ones column in v_sb).

Layout trick (as baseline): activations host-transposed to D-major with an
even/odd row permutation of d_model so interleaved-repeat RoPE tables
collapse to 512 distinct rows, partition-aligned in 128-chunks.
"""
import os
import sys
from collections import deque

for _p in ("/opt/trn_rl_repo", "/root/.axon_site/_ro/trn_rl_repo"):
    if os.path.isdir(_p) and _p not in sys.path:
        sys.path.insert(0, _p)

import numpy as np

import concourse.bacc as bacc
import concourse.tile as tile
import concourse.mybir as mybir
from concourse.bass_utils import run_bass_kernel_spmd

B, S, D = 2, 2048, 1024
H_TOT, DK = 16, 64
N_CORES, GROUPS = 8, 4
CH = 256            # channels (heads*dk) per core
KC = D // 128       # 8 d-model chunks
SB = 512            # seq block
NB = S // SB        # 4 seq blocks
ST = S // 128       # 16 seq tiles
HPC = 4             # heads per core
BASE = 10000.0
SPL = 416           # DVE/GpSimd elementwise split point within a 512 block

MM = mybir.dt.float16
F32 = mybir.dt.float32
AF = mybir.ActivationFunctionType

# even/odd permutation of the d_model axis: row r <- old d = 2r (r<512), 2(r-512)+1
_PERM = np.concatenate([np.arange(0, D, 2), np.arange(1, D, 2)])

_PROG = None


def _build():
    nc = bacc.Bacc("TRN2", target_bir_lowering=False, debug=False)
    qT = nc.dram_tensor("qT", (D, S), MM, kind="ExternalInput").ap()
    kT = nc.dram_tensor("kT", (D, S), MM, kind="ExternalInput").ap()
    vT = nc.dram_tensor("vT", (D, S), MM, kind="ExternalInput").ap()
    cosc = nc.dram_tensor("cosc", (D // 2, S), MM, kind="ExternalInput").ap()
    sinc = nc.dram_tensor("sinc", (D // 2, S), MM, kind="ExternalInput").ap()
    wq = nc.dram_tensor("wq", (D, CH), MM, kind="ExternalInput").ap()
    wk = nc.dram_tensor("wk", (D, CH), MM, kind="ExternalInput").ap()
    wv = nc.dram_tensor("wv", (D, CH), MM, kind="ExternalInput").ap()
    wo = nc.dram_tensor("wo", (CH, D), MM, kind="ExternalInput").ap()
    bq = nc.dram_tensor("bq", (2, 128, 1), F32, kind="ExternalInput").ap()
    bk = nc.dram_tensor("bk", (2, 128, 1), F32, kind="ExternalInput").ap()
    bvb = nc.dram_tensor("bvb", (128, CH), MM, kind="ExternalInput").ap()
    out = nc.dram_tensor("out", (S, D), MM, kind="ExternalOutput").ap()

    with tile.TileContext(nc) as tc:
      with (
          tc.tile_pool(name="consts", bufs=1) as consts,
          tc.tile_pool(name="qkv", bufs=1) as qkv,
          tc.tile_pool(name="rin", bufs=2) as rin,
          tc.tile_pool(name="expp", bufs=4) as expp,
          tc.tile_pool(name="misc", bufs=3) as misc,
          tc.tile_pool(name="outst", bufs=4) as outst,
          tc.tile_pool(name="ps_sc", bufs=2, space="PSUM") as ps_sc,
          tc.tile_pool(name="ps_ut", bufs=2, space="PSUM") as ps_ut,
          tc.tile_pool(name="ps_gen", bufs=2, space="PSUM") as ps_gen,
      ):
        # ---- ACT warmup: trigger the exp table-set load at t~0 ----
        wrm_i = consts.tile([1, 16], F32, tag="wrm_i")
        nc.vector.memset(wrm_i[:], 0.0)
        wrm_o = consts.tile([1, 16], F32, tag="wrm_o")
        nc.scalar.activation(wrm_o[:], wrm_i[:], AF.Exp)

        # ---- constants ----
        wq_sb = consts.tile([128, KC * CH], MM, tag="wq")
        wk_sb = consts.tile([128, KC * CH], MM, tag="wk")
        wv_sb = consts.tile([128, KC * CH], MM, tag="wv")
        wo_sb = consts.tile([128, 2 * D], MM, tag="wo")
        for c in range(KC):
            nc.sync.dma_start(wk_sb[:, CH * c:CH * (c + 1)], wk[128 * c:128 * (c + 1), :])
            nc.sync.dma_start(wq_sb[:, CH * c:CH * (c + 1)], wq[128 * c:128 * (c + 1), :])
        bq_sb, bk_sb = [], []
        for c in range(2):
            t_ = consts.tile([128, 1], F32, tag=f"bq{c}", name=f"bq{c}")
            nc.sync.dma_start(t_[:], bq[c])
            bq_sb.append(t_)
            t_ = consts.tile([128, 1], F32, tag=f"bk{c}", name=f"bk{c}")
            nc.sync.dma_start(t_[:], bk[c])
            bk_sb.append(t_)
        bvb_sb = consts.tile([128, CH], MM, tag="bvb")
        nc.sync.dma_start(bvb_sb[:], bvb)

        # V storage: per (tile t, head h) block of 65 cols: 64 values + ones col
        v_sb = qkv.tile([128, ST * HPC * 65], MM, tag="v")
        ones_cols = v_sb[:].rearrange("p (b c) -> p b c", c=65)[:, :, 64]
        nc.vector.memset(ones_cols, 1.0)

        qt_sb = [qkv.tile([128, S], MM, tag=f"qt{c}", name=f"qt{c}") for c in range(2)]
        kt_sb = [qkv.tile([128, S], MM, tag=f"kt{c}", name=f"kt{c}") for c in range(2)]
        ut_sb = [qkv.tile([128, S], MM, tag=f"ut{c}", name=f"ut{c}") for c in range(2)]

        # ---- streamed DMA helpers ----
        cos_t, sin_t = {}, {}      # (ca, sb) -> [128, SB]
        kin, qin = {}, {}          # (a, sb) -> [128, SB]
        vch = {}                   # (d, sb) -> [128, SB]

        def dma_tbl(sb):
            for ca in (0, 2, 1, 3):
                t_ = rin.tile([128, SB], MM, tag="tbl", name=f"cos{ca}_{sb}", bufs=32)
                nc.sync.dma_start(t_[:], cosc[128 * ca:128 * (ca + 1), SB * sb:SB * (sb + 1)])
                cos_t[(ca, sb)] = t_
                t_ = rin.tile([128, SB], MM, tag="tbl", name=f"sin{ca}_{sb}", bufs=32)
                nc.sync.dma_start(t_[:], sinc[128 * ca:128 * (ca + 1), SB * sb:SB * (sb + 1)])
                sin_t[(ca, sb)] = t_

        def dma_in(src_t, dst, sb, pfx):
            for a in (0, 2, 1, 3, 4, 6, 5, 7):
                t_ = rin.tile([128, SB], MM, tag="ri", name=f"{pfx}{a}_{sb}", bufs=24)
                nc.sync.dma_start(t_[:], src_t[128 * a:128 * (a + 1), SB * sb:SB * (sb + 1)])
                dst[(a, sb)] = t_

        def dma_v(sb):
            for d in range(KC):
                t_ = rin.tile([128, SB], MM, tag="vch", name=f"v{d}_{sb}", bufs=24)
                nc.sync.dma_start(t_[:], vT[128 * d:128 * (d + 1), SB * sb:SB * (sb + 1)])
                vch[(d, sb)] = t_

        # ---- compute helpers ----
        def _ew(op, o, a, b):
            getattr(nc.vector, op)(o[:, :SPL], a[:, :SPL], b[:, :SPL])
            getattr(nc.gpsimd, op)(o[:, SPL:], a[:, SPL:], b[:, SPL:])

        def rope_block(src, sb):
            roped = [None] * KC
            for a in (0, 1, 4, 5):
                b_ = a + 2
                xa, xb = src.pop((a, sb)), src.pop((b_, sb))
                ca, cb = a % 4, b_ % 4
                t1 = rin.tile([128, SB], MM, tag="tmp", name="t1", bufs=6)
                _ew("tensor_mul", t1, xa, cos_t[(ca, sb)])
                t2 = rin.tile([128, SB], MM, tag="tmp", name="t2", bufs=6)
                _ew("tensor_mul", t2, xb, sin_t[(ca, sb)])
                ra = rin.tile([128, SB], MM, tag="roped", name="ra", bufs=12)
                _ew("tensor_sub", ra, t1, t2)
                t3 = rin.tile([128, SB], MM, tag="tmp", name="t3", bufs=6)
                _ew("tensor_mul", t3, xb, cos_t[(cb, sb)])
                t4 = rin.tile([128, SB], MM, tag="tmp", name="t4", bufs=6)
                _ew("tensor_mul", t4, xa, sin_t[(cb, sb)])
                rb = rin.tile([128, SB], MM, tag="roped", name="rb", bufs=12)
                _ew("tensor_add", rb, t3, t4)
                roped[a], roped[b_] = ra, rb
            return roped

        def proj_half(roped, w_sb, b_sb, dst, sb, c):
            ps = ps_gen.tile([128, SB], F32, tag="gen", name="psp")
            for d in range(KC):
                lhsT = w_sb[:, CH * d + 128 * c: CH * d + 128 * (c + 1)]
                nc.tensor.matmul(ps[:], lhsT, roped[d][:],
                                 start=(d == 0), stop=(d == KC - 1))
            nc.vector.tensor_scalar_add(dst[c][:, SB * sb:SB * (sb + 1)], ps[:], b_sb[c][:])

        def vproj(t):
            sb, i = divmod(t, 4)
            ps = ps_gen.tile([128, CH], F32, tag="gen", name="psv")
            for d in range(KC):
                lhsT = vch[(d, sb)][:, 128 * i:128 * (i + 1)]
                nc.tensor.matmul(ps[:], lhsT, wv_sb[:, CH * d:CH * (d + 1)],
                                 start=(d == 0), stop=(d == KC - 1))
            dst = v_sb[:, t * HPC * 65: (t * HPC + HPC) * 65]
            dst = dst.rearrange("p (h j) -> p h j", h=HPC)[:, :, 0:64]
            nc.vector.tensor_add(dst, ps[:].rearrange("p (h j) -> p h j", h=HPC),
                                 bvb_sb[:].rearrange("p (h j) -> p h j", h=HPC))

        # ================= streamed prologue =================
        dma_tbl(0)
        dma_in(kT, kin, 0, "k")
        rk0 = rope_block(kin, 0)
        proj_half(rk0, wk_sb, bk_sb, kt_sb, 0, 0)
        proj_half(rk0, wk_sb, bk_sb, kt_sb, 0, 1)
        dma_in(qT, qin, 0, "q")
        rq0 = rope_block(qin, 0)
        proj_half(rq0, wq_sb, bq_sb, qt_sb, 0, 0)
        proj_half(rq0, wq_sb, bq_sb, qt_sb, 0, 1)

        dma_tbl(1)
        dma_in(kT, kin, 1, "k")
        rk1 = rope_block(kin, 1)
        proj_half(rk1, wk_sb, bk_sb, kt_sb, 1, 0)
        proj_half(rk1, wk_sb, bk_sb, kt_sb, 1, 1)

        dma_v(0)
        for c in range(KC):
            nc.sync.dma_start(wv_sb[:, CH * c:CH * (c + 1)], wv[128 * c:128 * (c + 1), :])
        for t in range(0, 4):
            vproj(t)

        dma_tbl(2)
        dma_in(kT, kin, 2, "k")
        rk2 = rope_block(kin, 2)
        proj_half(rk2, wk_sb, bk_sb, kt_sb, 2, 0)
        proj_half(rk2, wk_sb, bk_sb, kt_sb, 2, 1)

        dma_v(1)
        for t in range(4, 8):
            vproj(t)

        dma_tbl(3)
        dma_in(kT, kin, 3, "k")
        rk3 = rope_block(kin, 3)
        proj_half(rk3, wk_sb, bk_sb, kt_sb, 3, 0)
        proj_half(rk3, wk_sb, bk_sb, kt_sb, 3, 1)

        dma_in(qT, qin, 1, "q")
        dma_v(2)
        dma_in(qT, qin, 2, "q")
        dma_v(3)
        dma_in(qT, qin, 3, "q")
        for c in range(2):
            nc.sync.dma_start(wo_sb[:, D * c:D * (c + 1)], wo[128 * c:128 * (c + 1), :])

        # ================= attention with background thunks =================
        bg = deque()
        for t in range(8, 16):
            bg.append((lambda t=t: vproj(t)))

        def rope_proj_q(sb):
            def go(sb=sb):
                rq = rope_block(qin, sb)
                proj_half(rq, wq_sb, bq_sb, qt_sb, sb, 0)
                proj_half(rq, wq_sb, bq_sb, qt_sb, sb, 1)
            return go

        def outproj_st(st):
            def go(st=st):
                pos = [ps_gen.tile([128, 512], F32, tag="gen", name="po_")
                       for _ in range(2)]
                for cc in range(2):
                    lhsT = ut_sb[cc][:, 128 * st:128 * (st + 1)]
                    for nb in range(2):
                        nc.tensor.matmul(
                            pos[nb][:], lhsT,
                            wo_sb[:, D * cc + 512 * nb: D * cc + 512 * (nb + 1)],
                            start=(cc == 0), stop=(cc == 1))
                for nb in range(2):
                    stg = outst.tile([128, 512], MM, tag="stg", name="stg")
                    nc.vector.tensor_copy(stg[:], pos[nb][:])
                    nc.sync.dma_start(
                        out[128 * st:128 * (st + 1), 512 * nb:512 * (nb + 1)],
                        stg[:])
            return go

        for qb in range(NB):
            if qb < NB - 1:
                bg.append(rope_proj_q(qb + 1))
            for hp in range(2):
                puts = [ps_ut.tile([65, 512], F32, tag="put", name=f"put{half}")
                        for half in range(2)]
                for t in range(ST):
                    psc = ps_sc.tile([128, 1024], F32, tag="sc", name="psc")
                    for half in range(2):
                        base = 64 * half
                        nc.tensor.matmul(
                            psc[:, 512 * half:512 * (half + 1)],
                            kt_sb[hp][base:base + 64, 128 * t:128 * (t + 1)],
                            qt_sb[hp][base:base + 64, 512 * qb:512 * (qb + 1)],
                            start=True, stop=True)
                    e = expp.tile([128, 1024], MM, tag="e", name="e")
                    nc.scalar.activation(e[:], psc[:], AF.Exp, scale=0.125)
                    for half in range(2):
                        h = 2 * hp + half
                        vs = v_sb[:, (t * HPC + h) * 65:(t * HPC + h) * 65 + 65]
                        nc.tensor.matmul(puts[half][:], vs,
                                         e[:, 512 * half:512 * (half + 1)],
                                         start=(t == 0), stop=(t == ST - 1),
                                         skip_group_check=True)
                    if bg:
                        bg.popleft()()
                # normalize: U/denom (+ bv already folded into V)
                for half in range(2):
                    put_ = puts[half]
                    rec = misc.tile([1, 512], F32, tag="rec", name="rec")
                    nc.vector.reciprocal(rec[:], put_[64:65, :])
                    bc = misc.tile([64, 512], F32, tag="bc", name="bc")
                    nc.gpsimd.partition_broadcast(bc[:], rec[:])
                    dst = ut_sb[hp][64 * half:64 * (half + 1), 512 * qb:512 * (qb + 1)]
                    nc.vector.tensor_mul(dst, put_[0:64, :], bc[:])
            for st in range(4 * qb, 4 * qb + 4):
                bg.append(outproj_st(st))
        while bg:
            bg.popleft()()
    nc.compile()
    return nc


def _prepare(q, k, v, Wq_w, Wq_b, Wk_w, Wk_b, Wv_w, Wv_b, Wo_w, Wo_b):
    f16 = np.float16
    pos = np.arange(1, S + 1, dtype=np.float32)
    theta = (BASE ** (-2.0 * np.arange(D // 2, dtype=np.float32) / D)).astype(np.float32)
    ang = theta[:, None] * pos[None, :]
    cosc = np.cos(ang).astype(f16)
    sinc = np.sin(ang).astype(f16)

    per_batch = []
    for b in range(B):
        per_batch.append((
            np.ascontiguousarray(q[b].T[_PERM]).astype(f16),
            np.ascontiguousarray(k[b].T[_PERM]).astype(f16),
            np.ascontiguousarray(v[b].T).astype(f16),
        ))
    in_maps = []
    for c in range(N_CORES):
        b, g = divmod(c, GROUPS)
        rows = slice(CH * g, CH * (g + 1))
        qTb, kTb, vTb = per_batch[b]
        in_maps.append({
            "qT": qTb, "kT": kTb, "vT": vTb, "cosc": cosc, "sinc": sinc,
            "wq": np.ascontiguousarray(Wq_w[rows, :].T[_PERM]).astype(f16),
            "wk": np.ascontiguousarray(Wk_w[rows, :].T[_PERM]).astype(f16),
            "wv": np.ascontiguousarray(Wv_w[rows, :].T).astype(f16),
            "wo": np.ascontiguousarray(Wo_w[:, rows].T).astype(f16),
            "bq": Wq_b[rows].astype(np.float32).reshape(2, 128, 1),
            "bk": Wk_b[rows].astype(np.float32).reshape(2, 128, 1),
            "bvb": np.ascontiguousarray(
                np.tile(Wv_b[rows].astype(f16)[None, :], (128, 1))),
        })
    return in_maps


def kernel(q, k, v, Wq_w, Wq_b, Wk_w, Wk_b, Wv_w, Wv_b, Wo_w, Wo_b):
    global _PROG
    args = [np.asarray(x, dtype=np.float32) for x in
            (q, k, v, Wq_w, Wq_b, Wk_w, Wk_b, Wv_w, Wv_b, Wo_w, Wo_b)]
    if _PROG is None:
        _PROG = _build()
    in_maps = _prepare(*args)
    res = run_bass_kernel_spmd(_PROG, in_maps, core_ids=list(range(N_CORES)))
    kernel.last_results = res
    Wo_b32 = args[10]
    out = np.empty((B, S, D), dtype=np.float32)
    for b in range(B):
        acc = res.results[GROUPS * b]["out"].astype(np.float32)
        for g in range(1, GROUPS):
            acc += res.results[GROUPS * b + g]["out"]
        out[b] = acc + Wo_b32
    return out
